# revision 7
# baseline (speedup 1.0000x reference)
"""Causal self-attention with RoPE on 8 TRN2 NeuronCores.

Head-parallel tensor parallelism: core i owns heads 2i, 2i+1. Each core
computes its slice of the qkv projection, per-head causal attention
entirely in SBUF, and a partial output projection over its 128 channels;
a column-chunked ReduceScatter sums partials and leaves each core with
its 512-row shard of the output.

All matmuls run in float32r (full PE rate, ~tf32 mantissa). Erratum
rules respected: no f32r transpose / explicit tile_position, no mixed
base partitions inside one PSUM accumulation group.
"""

import numpy as np

import concourse.bass as bass
import concourse.mybir as mybir
import concourse.tile as tile
from concourse import bacc
from concourse.bass_utils import run_bass_kernel_spmd

F32 = mybir.dt.float32
F32R = mybir.dt.float32r

B, T, C = 2, 2048, 1024
H, HD = 16, 64
NC = 8
HL = H // NC          # heads per core = 2
BT = B * T            # 4096
FQKV = 3 * HL * HD    # 384 rows of w_attn per core
TSH = BT // NC        # 512 output rows per core
NCH = BT // 512       # 8 column chunks of the [*, BT] activations
ROPE_BASE = 10000.0


def build():
    nc = bacc.Bacc(None, target_bir_lowering=False)

    xT_d = nc.dram_tensor("xT", [C, BT], F32R, kind="ExternalInput")
    wq_d = nc.dram_tensor("wqkvT", [C, FQKV], F32R, kind="ExternalInput")
    wp0_d = nc.dram_tensor("wpT0", [HD, C], F32R, kind="ExternalInput")
    wp1_d = nc.dram_tensor("wpT1", [HD, C], F32R, kind="ExternalInput")
    cos_d = nc.dram_tensor("cosT", [128, BT], F32R, kind="ExternalInput")
    sin_d = nc.dram_tensor("sinT", [128, BT], F32R, kind="ExternalInput")
    perm_d = nc.dram_tensor("permT", [128, 128], F32R, kind="ExternalInput")
    mask_d = nc.dram_tensor("masks", [4, 128, 512], F32R, kind="ExternalInput")
    id_d = nc.dram_tensor("ident", [128, 128], F32, kind="ExternalInput")
    out_d = nc.dram_tensor("out", [TSH, C], F32, kind="ExternalOutput")

    partial_c = [nc.dram_tensor(f"partial{oc}", [BT, 512], F32)
                 for oc in range(2)]
    rs_c = [nc.dram_tensor(f"rs{oc}", [TSH, 512], F32) for oc in range(2)]

    with tile.TileContext(nc) as tc:
        with (
            tc.tile_pool(name="persist", bufs=1) as pp,
            tc.tile_pool(name="work", bufs=2) as wk,
            tc.tile_pool(name="pts", bufs=6) as ptp,
            tc.tile_pool(name="psA", bufs=2, space="PSUM") as psA,
            tc.tile_pool(name="psS", bufs=2, space="PSUM") as psS,
            tc.tile_pool(name="psV", bufs=1, space="PSUM") as psV,
        ):
            # ---- constants / weights (persist) ----
            wq_sb = []
            for c in range(8):
                t = pp.tile([128, FQKV], F32R, name=f"wq{c}", tag=f"wq{c}")
                nc.sync.dma_start(t[:], wq_d[c * 128:(c + 1) * 128, :])
                wq_sb.append(t)
            wp_sb = []
            for hidx, w_d in enumerate((wp0_d, wp1_d)):
                t = pp.tile([HD, C], F32R, name=f"wp{hidx}", tag=f"wp{hidx}")
                nc.sync.dma_start(t[:], w_d[:])
                wp_sb.append(t)
            perm_sb = pp.tile([128, 128], F32R, name="perm_sb", tag="perm_sb")
            nc.sync.dma_start(perm_sb[:], perm_d[:])
            id_sb = pp.tile([128, 128], F32, name="id_sb", tag="id_sb")
            nc.sync.dma_start(id_sb[:], id_d[:])
            mask_sb = []
            for m in range(4):
                t = pp.tile([128, 512], F32R, name=f"mask{m}", tag=f"mask{m}")
                nc.sync.dma_start(t[:], mask_d[m])
                mask_sb.append(t)
            ones_f = pp.tile([1, HD], F32, name="ones_f", tag="ones_f")
            nc.vector.memset(ones_f[:], 1.0)
            ones_c = pp.tile([128, 1], F32, name="ones_c", tag="ones_c")
            nc.vector.memset(ones_c[:], 1.0)

            # chunked activations: 8 chunks of [128, 512] each
            qtc = [pp.tile([128, 512], F32R, name=f"qtc{i}", tag=f"qtc{i}")
                   for i in range(NCH)]
            ktc = [pp.tile([128, 512], F32R, name=f"ktc{i}", tag=f"ktc{i}")
                   for i in range(NCH)]
            vtc = [pp.tile([128, 512], F32, name=f"vtc{i}", tag=f"vtc{i}")
                   for i in range(NCH)]
            fdst = [qtc, ktc, vtc]

            # ---- phase 1: qkvT = wqkvT.T @ xT, [f, t] layout ----
            for th in range(4):          # t quarters to bound xT residency
                xt_sb = []
                for c in range(8):
                    t = pp.tile([128, 1024], F32R, name=f"xt{th}{c}",
                                tag=f"xt{c}")
                    nc.sync.dma_start(t[:], xT_d[c * 128:(c + 1) * 128,
                                                 th * 1024:(th + 1) * 1024])
                    xt_sb.append(t)
                for f in range(3):
                    pq = [psA.tile([128, 512], F32, name=f"pq{th}{f}{tq}",
                                   tag="ps_a") for tq in range(2)]
                    for c in range(8):
                        for tq in range(2):
                            nc.tensor.matmul(
                                pq[tq][:],
                                wq_sb[c][:, f * 128:(f + 1) * 128],
                                xt_sb[c][:, tq * 512:(tq + 1) * 512],
                                start=(c == 0), stop=(c == 7),
                            )
                    for tq in range(2):
                        nc.scalar.copy(fdst[f][th * 2 + tq][:], pq[tq][:])

            # ---- phase 2: RoPE on q, k chunks (in place) ----
            for ch in range(NCH):
                cosc = wk.tile([128, 512], F32R, name=f"cosc{ch}", tag="cosc")
                nc.sync.dma_start(cosc[:], cos_d[:, ch * 512:(ch + 1) * 512])
                sinc = wk.tile([128, 512], F32R, name=f"sinc{ch}", tag="sinc")
                nc.sync.dma_start(sinc[:], sin_d[:, ch * 512:(ch + 1) * 512])
                for which, tcl in (("q", qtc), ("k", ktc)):
                    src = tcl[ch]
                    pr = psA.tile([128, 512], F32, name=f"pr{which}{ch}",
                                  tag="ps_a")
                    nc.tensor.matmul(pr[:], perm_sb[:], src[:],
                                     start=True, stop=True)
                    rot = wk.tile([128, 512], F32R, name=f"rot{which}{ch}",
                                  tag="rot")
                    nc.vector.tensor_mul(rot[:], pr[:], sinc[:])
                    nc.vector.tensor_mul(src[:], src[:], cosc[:])
                    nc.vector.tensor_add(src[:], src[:], rot[:])

            # ---- phase 3: V blocks [t, d] with ones columns ----
            v_sb = []
            for kb in range(BT // 128):  # 32 key blocks across both batches
                pv = psA.tile([128, 128], F32, name=f"pv{kb}", tag="ps_a")
                nc.tensor.transpose(
                    pv[:], vtc[kb // 4][:, (kb % 4) * 128:(kb % 4 + 1) * 128],
                    id_sb[:])
                v = pp.tile([128, 2 * (HD + 1)], F32R, name=f"v{kb}",
                            tag=f"v{kb}")
                nc.vector.tensor_copy(v[:, 0:HD], pv[:, 0:HD])
                nc.vector.tensor_copy(v[:, HD + 1:2 * HD + 1],
                                      pv[:, HD:2 * HD])
                nc.vector.tensor_copy(v[:, HD:HD + 1], ones_c[:])
                nc.vector.tensor_copy(v[:, 2 * HD + 1:2 * HD + 2], ones_c[:])
                v_sb.append(v)

            # ---- phase 4: attention per (batch, head), kb-outer ----
            atc = [[pp.tile([HD, 512], F32R, name=f"atc{h}_{i}",
                            tag=f"atc{h}_{i}") for i in range(NCH)]
                   for h in range(HL)]

            for b in range(B):
                for h in range(HL):
                    hp = h * 64
                    avp = psV.tile([HD + 1, 2048], F32, name=f"av{b}{h}",
                                   tag="ps_av")

                    def s_group(kb, b=b, h=h, hp=hp):
                        """S^T + exp (+mask) for all valid q chunks of kb."""
                        res = []
                        kch = ktc[b * 4 + kb // 4]
                        koff = (kb % 4) * 128
                        for qc in range(kb // 4, 4):
                            sps = psS.tile([128, 512], F32,
                                           name=f"s{b}{h}{kb}{qc}",
                                           tag="ps_s")
                            nc.tensor.matmul(
                                sps[:],
                                kch[hp:hp + 64, koff:koff + 128],
                                qtc[b * 4 + qc][hp:hp + 64, :],
                                start=True, stop=True,
                            )
                            pt = ptp.tile([128, 512], F32R,
                                          name=f"pt{b}{h}{kb}{qc}", tag="pt")
                            nc.scalar.activation(
                                pt[:], sps[:],
                                mybir.ActivationFunctionType.Exp,
                                scale=0.125,
                            )
                            if qc == kb // 4:
                                nc.vector.tensor_mul(
                                    pt[:], pt[:], mask_sb[kb % 4][:])
                            res.append((qc, pt))
                        return res

                    def av_group(kb, pts, b=b, h=h, avp=avp):
                        for qc, pt in pts:
                            nc.tensor.matmul(
                                avp[:, qc * 512:(qc + 1) * 512],
                                v_sb[b * 16 + kb][:, h * (HD + 1):
                                                  (h + 1) * (HD + 1)],
                                pt[:],
                                start=(kb == 0), stop=(kb == 4 * qc + 3),
                            )

                    # software-pipeline S one kb-group ahead of AV
                    prev = s_group(0)
                    for kb in range(1, 16):
                        cur = s_group(kb)
                        av_group(kb - 1, prev)
                        prev = cur
                    av_group(15, prev)

                    for qc in range(4):
                        qsl = slice(qc * 512, (qc + 1) * 512)
                        den = wk.tile([HD + 1, 512], F32,
                                      name=f"den{b}{h}{qc}", tag="den")
                        nc.scalar.copy(den[HD:HD + 1, :], avp[HD:HD + 1, qsl])
                        den0 = wk.tile([1, 512], F32, name=f"den0{b}{h}{qc}",
                                       tag="den0")
                        nc.sync.dma_start(den0[:], den[HD:HD + 1, :])
                        rcp0 = wk.tile([1, 512], F32, name=f"rcp0{b}{h}{qc}",
                                       tag="rcp0")
                        scr = wk.tile([1, 512], F32, name=f"scr{b}{h}{qc}",
                                      tag="scr")
                        nc.vector.reciprocal_approx_accurate(
                            out=rcp0[:], in_=den0[:], scratch=scr[:])
                        pbc = psS.tile([HD, 512], F32, name=f"pbc{b}{h}{qc}",
                                       tag="ps_s")
                        nc.tensor.matmul(pbc[:], ones_f[:], rcp0[:],
                                         start=True, stop=True)
                        bc = wk.tile([HD, 512], F32, name=f"bc{b}{h}{qc}",
                                     tag="bc")
                        nc.scalar.copy(bc[:], pbc[:])
                        nc.vector.tensor_mul(atc[h][b * 4 + qc][:],
                                             avp[0:HD, qsl], bc[:])

            # ---- phase 5: partial out-proj, column-chunked + RS ----
            for oc in range(2):
                osl = slice(oc * 512, (oc + 1) * 512)
                for tb in range(BT // 128):
                    po = psA.tile([128, 512], F32, name=f"po{oc}{tb}",
                                  tag="ps_a")
                    for h in range(HL):
                        nc.tensor.matmul(
                            po[:],
                            atc[h][tb // 4][:, (tb % 4) * 128:
                                            (tb % 4 + 1) * 128],
                            wp_sb[h][:, osl],
                            start=(h == 0), stop=(h == HL - 1),
                        )
                    st = wk.tile([128, 512], F32, name=f"st{oc}{tb}", tag="st")
                    nc.scalar.copy(st[:], po[:])
                    nc.sync.dma_start(
                        partial_c[oc][tb * 128:(tb + 1) * 128, :], st[:])
                nc.gpsimd.collective_compute(
                    "ReduceScatter",
                    mybir.AluOpType.add,
                    replica_groups=[list(range(NC))],
                    ins=[partial_c[oc][:]],
                    outs=[rs_c[oc][:]],
                )
                nc.sync.dma_start(out_d[:, osl], rs_c[oc][:])

    nc.finalize()
    return nc


def host_inputs(x, w_attn, w_proj):
    """Host-side sharding/layout prep. Returns per-core in_maps."""
    x2 = np.ascontiguousarray(x.reshape(BT, C).T).astype(np.float32)  # [C,BT]

    inv = 1.0 / (ROPE_BASE ** (np.arange(0, HD, 2, dtype=np.float32) / HD))
    tpos = np.arange(T, dtype=np.float32)
    freqs = tpos[:, None] * inv[None, :]                  # [T, 32]
    emb = np.concatenate([freqs, freqs], axis=-1)         # [T, 64]
    cosT = np.cos(emb).T.astype(np.float32)               # [64, T]
    sinT = np.sin(emb).T.astype(np.float32)
    cos_full = np.ascontiguousarray(np.tile(cosT, (2, B)))  # [128, BT]
    sin_full = np.ascontiguousarray(np.tile(sinT, (2, B)))

    m64 = np.zeros((HD, HD), dtype=np.float32)
    half = HD // 2
    for d in range(half):
        m64[d, d + half] = -1.0
        m64[d + half, d] = 1.0
    perm = np.zeros((128, 128), dtype=np.float32)
    perm[0:HD, 0:HD] = m64
    perm[HD:128, HD:128] = m64
    permT = np.ascontiguousarray(perm.T)

    masks = np.zeros((4, 128, 512), dtype=np.float32)
    qi = np.arange(512)[None, :]
    ki = np.arange(128)[:, None]
    for m in range(4):
        masks[m] = (qi - ki >= m * 128).astype(np.float32)

    ident = np.eye(128, dtype=np.float32)

    in_maps = []
    for i in range(NC):
        r0 = i * (HL * HD)
        wq = w_attn[r0:r0 + HL * HD, :]
        wk_ = w_attn[C + r0:C + r0 + HL * HD, :]
        wv = w_attn[2 * C + r0:2 * C + r0 + HL * HD, :]
        wqkvT = np.ascontiguousarray(
            np.concatenate([wq, wk_, wv], axis=0).T).astype(np.float32)
        c0 = i * (HL * HD)
        wpT0 = np.ascontiguousarray(w_proj[:, c0:c0 + HD].T).astype(np.float32)
        wpT1 = np.ascontiguousarray(
            w_proj[:, c0 + HD:c0 + 2 * HD].T).astype(np.float32)
        in_maps.append({
            "xT": x2, "wqkvT": wqkvT, "wpT0": wpT0, "wpT1": wpT1,
            "cosT": cos_full, "sinT": sin_full, "permT": permT,
            "masks": masks, "ident": ident,
        })
    return in_maps


_NC_CACHE = None


def _get_nc():
    global _NC_CACHE
    if _NC_CACHE is None:
        _NC_CACHE = build()
    return _NC_CACHE


def run(x, w_attn, w_proj, trace=False):
    nc = _get_nc()
    in_maps = host_inputs(np.asarray(x), np.asarray(w_attn),
                          np.asarray(w_proj))
    res = run_bass_kernel_spmd(nc, in_maps, list(range(NC)), trace=trace)
    shards = [res.results[i]["out"] for i in range(NC)]
    out = np.concatenate(shards, axis=0).reshape(B, T, C)
    return out.astype(np.float32), res


def kernel(x, w_attn, w_proj):
    out, _ = run(x, w_attn, w_proj, trace=False)
    return out


# revision 8
# speedup vs baseline: 1.0391x; 1.0391x over previous
"""Causal self-attention with RoPE on 8 TRN2 NeuronCores.

Head-parallel tensor parallelism: core i owns heads 2i, 2i+1. Each core
computes its slice of the qkv projection, per-head causal attention
entirely in SBUF, and a partial output projection over its 128 channels;
a column-chunked ReduceScatter sums partials and leaves each core with
its 512-row shard of the output.

All matmuls run in float32r (full PE rate, ~tf32 mantissa). Erratum
rules respected: no f32r transpose / explicit tile_position, no mixed
base partitions inside one PSUM accumulation group.
"""

import numpy as np

import concourse.bass as bass
import concourse.mybir as mybir
import concourse.tile as tile
from concourse import bacc
from concourse.bass_utils import run_bass_kernel_spmd

F32 = mybir.dt.float32
F32R = mybir.dt.float32r

B, T, C = 2, 2048, 1024
H, HD = 16, 64
NC = 8
HL = H // NC          # heads per core = 2
BT = B * T            # 4096
FQKV = 3 * HL * HD    # 384 rows of w_attn per core
TSH = BT // NC        # 512 output rows per core
NCH = BT // 512       # 8 column chunks of the [*, BT] activations
ROPE_BASE = 10000.0


def build():
    nc = bacc.Bacc(None, target_bir_lowering=False)

    xT_d = nc.dram_tensor("xT", [C, BT], F32R, kind="ExternalInput")
    wq_d = nc.dram_tensor("wqkvT", [C, FQKV], F32R, kind="ExternalInput")
    wp_d = nc.dram_tensor("wpT", [128, C], F32R, kind="ExternalInput")
    cos_d = nc.dram_tensor("cosT", [128, BT], F32R, kind="ExternalInput")
    sin_d = nc.dram_tensor("sinT", [128, BT], F32R, kind="ExternalInput")
    perm_d = nc.dram_tensor("permT", [128, 128], F32R, kind="ExternalInput")
    mask_d = nc.dram_tensor("masks", [4, 128, 512], F32R, kind="ExternalInput")
    id_d = nc.dram_tensor("ident", [128, 128], F32, kind="ExternalInput")
    out_d = nc.dram_tensor("out", [TSH, C], F32, kind="ExternalOutput")

    partial_c = [nc.dram_tensor(f"partial{j}", [BT, 256], F32)
                 for j in range(4)]
    rs_c = [nc.dram_tensor(f"rs{j}", [TSH, 256], F32) for j in range(4)]

    with tile.TileContext(nc) as tc:
        with (
            tc.tile_pool(name="persist", bufs=1) as pp,
            tc.tile_pool(name="work", bufs=2) as wk,
            tc.tile_pool(name="pts", bufs=10) as ptp,
            tc.tile_pool(name="psA", bufs=2, space="PSUM") as psA,
            tc.tile_pool(name="psS", bufs=2, space="PSUM") as psS,
            tc.tile_pool(name="psV", bufs=1, space="PSUM") as psV,
        ):
            # ---- constants / weights (persist) ----
            wq_sb = []
            for c in range(8):
                t = pp.tile([128, FQKV], F32R, name=f"wq{c}", tag=f"wq{c}")
                nc.sync.dma_start(t[:], wq_d[c * 128:(c + 1) * 128, :])
                wq_sb.append(t)
            wp_sb = pp.tile([128, C], F32R, name="wp_sb", tag="wp_sb")
            nc.gpsimd.dma_start(wp_sb[:], wp_d[:])
            perm_sb = pp.tile([128, 128], F32R, name="perm_sb", tag="perm_sb")
            nc.gpsimd.dma_start(perm_sb[:], perm_d[:])
            id_sb = pp.tile([128, 128], F32, name="id_sb", tag="id_sb")
            nc.gpsimd.dma_start(id_sb[:], id_d[:])
            mask_sb = []
            for m in range(4):
                t = pp.tile([128, 512], F32R, name=f"mask{m}", tag=f"mask{m}")
                nc.gpsimd.dma_start(t[:], mask_d[m])
                mask_sb.append(t)
            ones_c = pp.tile([128, 1], F32, name="ones_c", tag="ones_c")
            nc.vector.memset(ones_c[:], 1.0)

            # chunked activations: 8 chunks of [128, 512] each
            qtc = [pp.tile([128, 512], F32R, name=f"qtc{i}", tag=f"qtc{i}")
                   for i in range(NCH)]
            ktc = [pp.tile([128, 512], F32R, name=f"ktc{i}", tag=f"ktc{i}")
                   for i in range(NCH)]
            vtc = [pp.tile([128, 512], F32, name=f"vtc{i}", tag=f"vtc{i}")
                   for i in range(NCH)]
            fdst = [qtc, ktc, vtc]

            # ---- phase 1: qkvT = wqkvT.T @ xT, [f, t] layout ----
            for th in range(4):          # t quarters to bound xT residency
                xt_sb = []
                for c in range(8):
                    t = pp.tile([128, 1024], F32R, name=f"xt{th}{c}",
                                tag=f"xt{c}")
                    nc.sync.dma_start(t[:], xT_d[c * 128:(c + 1) * 128,
                                                 th * 1024:(th + 1) * 1024])
                    xt_sb.append(t)
                for f in range(3):
                    pq = [psA.tile([128, 512], F32, name=f"pq{th}{f}{tq}",
                                   tag="ps_a") for tq in range(2)]
                    for c in range(8):
                        for tq in range(2):
                            nc.tensor.matmul(
                                pq[tq][:],
                                wq_sb[c][:, f * 128:(f + 1) * 128],
                                xt_sb[c][:, tq * 512:(tq + 1) * 512],
                                start=(c == 0), stop=(c == 7),
                            )
                    for tq in range(2):
                        nc.scalar.copy(fdst[f][th * 2 + tq][:], pq[tq][:])

            # ---- phase 2: RoPE on q, k chunks (in place) ----
            for ch in range(NCH):
                cosc = wk.tile([128, 512], F32R, name=f"cosc{ch}", tag="cosc")
                nc.gpsimd.dma_start(cosc[:], cos_d[:, ch * 512:(ch + 1) * 512])
                sinc = wk.tile([128, 512], F32R, name=f"sinc{ch}", tag="sinc")
                nc.gpsimd.dma_start(sinc[:], sin_d[:, ch * 512:(ch + 1) * 512])
                for which, tcl in (("q", qtc), ("k", ktc)):
                    src = tcl[ch]
                    pr = psA.tile([128, 512], F32, name=f"pr{which}{ch}",
                                  tag="ps_a")
                    nc.tensor.matmul(pr[:], perm_sb[:], src[:],
                                     start=True, stop=True)
                    rot = wk.tile([128, 512], F32R, name=f"rot{which}{ch}",
                                  tag="rot")
                    nc.vector.tensor_mul(rot[:], pr[:], sinc[:])
                    nc.vector.tensor_mul(src[:], src[:], cosc[:])
                    nc.vector.tensor_add(src[:], src[:], rot[:])

            # ---- phase 3: V blocks [t, d] with ones columns ----
            v_sb = []
            for kb in range(BT // 128):  # 32 key blocks across both batches
                pv = psA.tile([128, 128], F32, name=f"pv{kb}", tag="ps_a")
                nc.tensor.transpose(
                    pv[:], vtc[kb // 4][:, (kb % 4) * 128:(kb % 4 + 1) * 128],
                    id_sb[:])
                v = pp.tile([128, 2 * (HD + 1)], F32R, name=f"v{kb}",
                            tag=f"v{kb}")
                nc.vector.tensor_copy(v[:, 0:HD], pv[:, 0:HD])
                nc.vector.tensor_copy(v[:, HD + 1:2 * HD + 1],
                                      pv[:, HD:2 * HD])
                nc.vector.tensor_copy(v[:, HD:HD + 1], ones_c[:])
                nc.vector.tensor_copy(v[:, 2 * HD + 1:2 * HD + 2], ones_c[:])
                v_sb.append(v)

            # ---- phase 4: attention per (batch, head), kb-outer ----
            # combined per-chunk attention tiles: h0 rows 0:64, h1 rows 64:128
            atc = [pp.tile([128, 512], F32R, name=f"atc{i}", tag=f"atc{i}")
                   for i in range(NCH)]

            for b in range(B):
                for h in range(HL):
                    hp = h * 64
                    avp = psV.tile([HD + 1, 2048], F32, name=f"av{b}{h}",
                                   tag="ps_av")

                    def s_group(kb, b=b, h=h, hp=hp):
                        """S^T + exp (+mask) for all valid q chunks of kb."""
                        res = []
                        kch = ktc[b * 4 + kb // 4]
                        koff = (kb % 4) * 128
                        for qc in range(kb // 4, 4):
                            sps = psS.tile([128, 512], F32,
                                           name=f"s{b}{h}{kb}{qc}",
                                           tag="ps_s")
                            nc.tensor.matmul(
                                sps[:],
                                kch[hp:hp + 64, koff:koff + 128],
                                qtc[b * 4 + qc][hp:hp + 64, :],
                                start=True, stop=True,
                            )
                            pt = ptp.tile([128, 512], F32R,
                                          name=f"pt{b}{h}{kb}{qc}", tag="pt")
                            nc.scalar.activation(
                                pt[:], sps[:],
                                mybir.ActivationFunctionType.Exp,
                                scale=0.125,
                            )
                            if qc == kb // 4:
                                nc.vector.tensor_mul(
                                    pt[:], pt[:], mask_sb[kb % 4][:])
                            res.append((qc, pt))
                        return res

                    def av_group(kb, pts, b=b, h=h, avp=avp):
                        for qc, pt in pts:
                            nc.tensor.matmul(
                                avp[:, qc * 512:(qc + 1) * 512],
                                v_sb[b * 16 + kb][:, h * (HD + 1):
                                                  (h + 1) * (HD + 1)],
                                pt[:],
                                start=(kb == 0), stop=(kb == 4 * qc + 3),
                            )

                    # software-pipeline S one kb-group ahead of AV
                    prev = s_group(0)
                    for kb in range(1, 16):
                        cur = s_group(kb)
                        av_group(kb - 1, prev)
                        prev = cur
                    av_group(15, prev)

                    for qc in range(4):
                        qsl = slice(qc * 512, (qc + 1) * 512)
                        den = wk.tile([HD + 1, 512], F32,
                                      name=f"den{b}{h}{qc}", tag="den")
                        nc.scalar.copy(den[HD:HD + 1, :], avp[HD:HD + 1, qsl])
                        den0 = wk.tile([1, 512], F32, name=f"den0{b}{h}{qc}",
                                       tag="den0")
                        nc.sync.dma_start(den0[:], den[HD:HD + 1, :])
                        rcp0 = wk.tile([1, 512], F32, name=f"rcp0{b}{h}{qc}",
                                       tag="rcp0")
                        scr = wk.tile([1, 512], F32, name=f"scr{b}{h}{qc}",
                                      tag="scr")
                        nc.vector.reciprocal_approx_accurate(
                            out=rcp0[:], in_=den0[:], scratch=scr[:])
                        bc = wk.tile([HD, 512], F32, name=f"bc{b}{h}{qc}",
                                     tag="bc")
                        nc.gpsimd.partition_broadcast(bc[:], rcp0[:])
                        if h == 0:
                            nc.vector.tensor_mul(atc[b * 4 + qc][0:HD, :],
                                                 avp[0:HD, qsl], bc[:])
                        else:
                            ath1 = wk.tile([HD, 512], F32R,
                                           name=f"ath1{b}{qc}", tag="ath1")
                            nc.vector.tensor_mul(ath1[:], avp[0:HD, qsl],
                                                 bc[:])
                            nc.sync.dma_start(atc[b * 4 + qc][HD:128, :],
                                              ath1[:])

            # ---- phase 5: partial out-proj, column-chunked + RS ----
            for oc in range(2):
                osl = slice(oc * 512, (oc + 1) * 512)
                for tb in range(BT // 128):
                    po = psA.tile([128, 512], F32, name=f"po{oc}{tb}",
                                  tag="ps_a")
                    nc.tensor.matmul(
                        po[:],
                        atc[tb // 4][:, (tb % 4) * 128:(tb % 4 + 1) * 128],
                        wp_sb[:, osl],
                        start=True, stop=True,
                    )
                    st = wk.tile([128, 512], F32, name=f"st{oc}{tb}", tag="st")
                    nc.scalar.copy(st[:], po[:])
                    for j in range(2):
                        nc.sync.dma_start(
                            partial_c[2 * oc + j][tb * 128:(tb + 1) * 128, :],
                            st[:, j * 256:(j + 1) * 256])
                for j in range(2):
                    jj = 2 * oc + j
                    nc.gpsimd.collective_compute(
                        "ReduceScatter",
                        mybir.AluOpType.add,
                        replica_groups=[list(range(NC))],
                        ins=[partial_c[jj][:]],
                        outs=[rs_c[jj][:]],
                    )
                    nc.sync.dma_start(out_d[:, jj * 256:(jj + 1) * 256],
                                      rs_c[jj][:])

    nc.finalize()
    return nc


def host_inputs(x, w_attn, w_proj):
    """Host-side sharding/layout prep. Returns per-core in_maps."""
    x2 = np.ascontiguousarray(x.reshape(BT, C).T).astype(np.float32)  # [C,BT]

    inv = 1.0 / (ROPE_BASE ** (np.arange(0, HD, 2, dtype=np.float32) / HD))
    tpos = np.arange(T, dtype=np.float32)
    freqs = tpos[:, None] * inv[None, :]                  # [T, 32]
    emb = np.concatenate([freqs, freqs], axis=-1)         # [T, 64]
    cosT = np.cos(emb).T.astype(np.float32)               # [64, T]
    sinT = np.sin(emb).T.astype(np.float32)
    cos_full = np.ascontiguousarray(np.tile(cosT, (2, B)))  # [128, BT]
    sin_full = np.ascontiguousarray(np.tile(sinT, (2, B)))

    m64 = np.zeros((HD, HD), dtype=np.float32)
    half = HD // 2
    for d in range(half):
        m64[d, d + half] = -1.0
        m64[d + half, d] = 1.0
    perm = np.zeros((128, 128), dtype=np.float32)
    perm[0:HD, 0:HD] = m64
    perm[HD:128, HD:128] = m64
    permT = np.ascontiguousarray(perm.T)

    masks = np.zeros((4, 128, 512), dtype=np.float32)
    qi = np.arange(512)[None, :]
    ki = np.arange(128)[:, None]
    for m in range(4):
        masks[m] = (qi - ki >= m * 128).astype(np.float32)

    ident = np.eye(128, dtype=np.float32)

    in_maps = []
    for i in range(NC):
        r0 = i * (HL * HD)
        wq = w_attn[r0:r0 + HL * HD, :]
        wk_ = w_attn[C + r0:C + r0 + HL * HD, :]
        wv = w_attn[2 * C + r0:2 * C + r0 + HL * HD, :]
        wqkvT = np.ascontiguousarray(
            np.concatenate([wq, wk_, wv], axis=0).T).astype(np.float32)
        c0 = i * (HL * HD)
        wpT = np.ascontiguousarray(
            w_proj[:, c0:c0 + 2 * HD].T).astype(np.float32)
        in_maps.append({
            "xT": x2, "wqkvT": wqkvT, "wpT": wpT,
            "cosT": cos_full, "sinT": sin_full, "permT": permT,
            "masks": masks, "ident": ident,
        })
    return in_maps


_NC_CACHE = None


def _get_nc():
    global _NC_CACHE
    if _NC_CACHE is None:
        _NC_CACHE = build()
    return _NC_CACHE


def run(x, w_attn, w_proj, trace=False):
    nc = _get_nc()
    in_maps = host_inputs(np.asarray(x), np.asarray(w_attn),
                          np.asarray(w_proj))
    res = run_bass_kernel_spmd(nc, in_maps, list(range(NC)), trace=trace)
    shards = [res.results[i]["out"] for i in range(NC)]
    out = np.concatenate(shards, axis=0).reshape(B, T, C)
    return out.astype(np.float32), res


def kernel(x, w_attn, w_proj):
    out, _ = run(x, w_attn, w_proj, trace=False)
    return out


# revision 11
# speedup vs baseline: 1.2291x; 1.1828x over previous
"""Causal self-attention with RoPE on 8 TRN2 NeuronCores.

Head-parallel tensor parallelism: core i owns heads 2i, 2i+1. Each core
computes its slice of the qkv projection, per-head causal attention
entirely in SBUF, and a partial output projection over its 128 channels;
a column-chunked ReduceScatter sums partials and leaves each core with
its 512-row shard of the output.

All matmuls run in float32r (full PE rate, ~tf32 mantissa). Erratum
rules respected: no f32r transpose / explicit tile_position, no mixed
base partitions inside one PSUM accumulation group.
"""

import numpy as np

import concourse.bass as bass
import concourse.mybir as mybir
import concourse.tile as tile
from concourse import bacc
from concourse.bass_utils import run_bass_kernel_spmd

F32 = mybir.dt.float32
F32R = mybir.dt.float32r
BF16 = mybir.dt.bfloat16

B, T, C = 2, 2048, 1024
H, HD = 16, 64
NC = 8
HL = H // NC          # heads per core = 2
BT = B * T            # 4096
FQKV = 3 * HL * HD    # 384 rows of w_attn per core
TSH = BT // NC        # 512 output rows per core
NCH = BT // 512       # 8 column chunks of the [*, BT] activations
ROPE_BASE = 10000.0


def build():
    nc = bacc.Bacc(None, target_bir_lowering=False)

    xT_d = nc.dram_tensor("xT", [C, BT], F32R, kind="ExternalInput")
    wq_d = nc.dram_tensor("wqkvT", [C, FQKV], F32R, kind="ExternalInput")
    wp_d = nc.dram_tensor("wpT", [128, C], F32R, kind="ExternalInput")
    cos_d = nc.dram_tensor("cosT", [128, BT], F32R, kind="ExternalInput")
    sin_d = nc.dram_tensor("sinT", [128, BT], F32R, kind="ExternalInput")
    perm_d = nc.dram_tensor("permT", [128, 128], F32R, kind="ExternalInput")
    mask_d = nc.dram_tensor("masks", [4, 128, 512], F32R, kind="ExternalInput")
    id_d = nc.dram_tensor("ident", [128, 128], F32, kind="ExternalInput")
    out_d = nc.dram_tensor("out", [TSH, C], F32, kind="ExternalOutput")

    # chunk (b, j): batch b, columns [512j, 512j+512), bf16 to halve RS bytes
    partial_c = {(b_, j): nc.dram_tensor(f"partial{b_}{j}", [T, 512], BF16)
                 for b_ in range(2) for j in range(2)}
    rs_c = {(b_, j): nc.dram_tensor(f"rs{b_}{j}", [T // NC, 512], BF16)
            for b_ in range(2) for j in range(2)}

    with tile.TileContext(nc) as tc:
        with (
            tc.tile_pool(name="persist", bufs=1) as pp,
            tc.tile_pool(name="work", bufs=2) as wk,
            tc.tile_pool(name="pts", bufs=10) as ptp,
            tc.tile_pool(name="psA", bufs=2, space="PSUM") as psA,
            tc.tile_pool(name="psS", bufs=2, space="PSUM") as psS,
            tc.tile_pool(name="psV", bufs=1, space="PSUM") as psV,
        ):
            # ---- constants / weights (persist) ----
            wq_sb = []
            for c in range(8):
                t = pp.tile([128, FQKV], F32R, name=f"wq{c}", tag=f"wq{c}")
                nc.sync.dma_start(t[:], wq_d[c * 128:(c + 1) * 128, :])
                wq_sb.append(t)
            wp_sb = pp.tile([128, C], F32R, name="wp_sb", tag="wp_sb")
            nc.gpsimd.dma_start(wp_sb[:], wp_d[:])
            perm_sb = pp.tile([128, 128], F32R, name="perm_sb", tag="perm_sb")
            nc.gpsimd.dma_start(perm_sb[:], perm_d[:])
            id_sb = pp.tile([128, 128], F32, name="id_sb", tag="id_sb")
            nc.gpsimd.dma_start(id_sb[:], id_d[:])
            mask_sb = []
            for m in range(4):
                t = pp.tile([128, 512], F32R, name=f"mask{m}", tag=f"mask{m}")
                nc.gpsimd.dma_start(t[:], mask_d[m])
                mask_sb.append(t)
            ones_c = pp.tile([128, 1], F32, name="ones_c", tag="ones_c")
            nc.vector.memset(ones_c[:], 1.0)

            # chunked activations: 8 chunks of [128, 512] each
            qtc = [pp.tile([128, 512], F32R, name=f"qtc{i}", tag=f"qtc{i}")
                   for i in range(NCH)]
            ktc = [pp.tile([128, 512], F32R, name=f"ktc{i}", tag=f"ktc{i}")
                   for i in range(NCH)]
            vtc = [pp.tile([128, 512], F32, name=f"vtc{i}", tag=f"vtc{i}")
                   for i in range(NCH)]
            fdst = [qtc, ktc, vtc]

            # ---- phase 1: qkvT = wqkvT.T @ xT, [f, t] layout ----
            for th in range(4):          # t quarters to bound xT residency
                xt_sb = []
                for c in range(8):
                    t = pp.tile([128, 1024], F32R, name=f"xt{th}{c}",
                                tag=f"xt{c}")
                    nc.sync.dma_start(t[:], xT_d[c * 128:(c + 1) * 128,
                                                 th * 1024:(th + 1) * 1024])
                    xt_sb.append(t)
                for f in range(3):
                    pq = [psA.tile([128, 512], F32, name=f"pq{th}{f}{tq}",
                                   tag="ps_a") for tq in range(2)]
                    for c in range(8):
                        for tq in range(2):
                            nc.tensor.matmul(
                                pq[tq][:],
                                wq_sb[c][:, f * 128:(f + 1) * 128],
                                xt_sb[c][:, tq * 512:(tq + 1) * 512],
                                start=(c == 0), stop=(c == 7),
                            )
                    for tq in range(2):
                        nc.scalar.copy(fdst[f][th * 2 + tq][:], pq[tq][:])

            # ---- phase 2: RoPE on q, k chunks (in place) ----
            for ch in range(NCH):
                cosc = wk.tile([128, 512], F32R, name=f"cosc{ch}", tag="cosc")
                nc.gpsimd.dma_start(cosc[:], cos_d[:, ch * 512:(ch + 1) * 512])
                sinc = wk.tile([128, 512], F32R, name=f"sinc{ch}", tag="sinc")
                nc.gpsimd.dma_start(sinc[:], sin_d[:, ch * 512:(ch + 1) * 512])
                for which, tcl in (("q", qtc), ("k", ktc)):
                    src = tcl[ch]
                    pr = psA.tile([128, 512], F32, name=f"pr{which}{ch}",
                                  tag="ps_a")
                    nc.tensor.matmul(pr[:], perm_sb[:], src[:],
                                     start=True, stop=True)
                    rot = wk.tile([128, 512], F32R, name=f"rot{which}{ch}",
                                  tag="rot")
                    nc.vector.tensor_mul(rot[:], pr[:], sinc[:])
                    nc.vector.tensor_mul(src[:], src[:], cosc[:])
                    nc.vector.tensor_add(src[:], src[:], rot[:])

            # ---- phase 3: V blocks [t, d] with ones columns ----
            v_sb = []
            for kb in range(BT // 128):  # 32 key blocks across both batches
                pv = psA.tile([128, 128], F32, name=f"pv{kb}", tag="ps_a")
                nc.tensor.transpose(
                    pv[:], vtc[kb // 4][:, (kb % 4) * 128:(kb % 4 + 1) * 128],
                    id_sb[:])
                v = pp.tile([128, 2 * (HD + 1)], F32R, name=f"v{kb}",
                            tag=f"v{kb}")
                nc.vector.tensor_copy(v[:, 0:HD], pv[:, 0:HD])
                nc.vector.tensor_copy(v[:, HD + 1:2 * HD + 1],
                                      pv[:, HD:2 * HD])
                nc.vector.tensor_copy(v[:, HD:HD + 1], ones_c[:])
                nc.vector.tensor_copy(v[:, 2 * HD + 1:2 * HD + 2], ones_c[:])
                v_sb.append(v)

            # ---- phase 4: attention per (batch, head), kb-outer ----
            # combined per-chunk attention tiles: h0 rows 0:64, h1 rows 64:128
            atc = [pp.tile([128, 512], F32R, name=f"atc{i}", tag=f"atc{i}")
                   for i in range(NCH)]

            def attention(b):
                for h in range(HL):
                    hp = h * 64
                    avp = psV.tile([HD + 1, 2048], F32, name=f"av{b}{h}",
                                   tag="ps_av")

                    def s_group(kb, b=b, h=h, hp=hp):
                        """S^T + exp (+mask) for all valid q chunks of kb."""
                        res = []
                        kch = ktc[b * 4 + kb // 4]
                        koff = (kb % 4) * 128
                        for qc in range(kb // 4, 4):
                            sps = psS.tile([128, 512], F32,
                                           name=f"s{b}{h}{kb}{qc}",
                                           tag="ps_s")
                            nc.tensor.matmul(
                                sps[:],
                                kch[hp:hp + 64, koff:koff + 128],
                                qtc[b * 4 + qc][hp:hp + 64, :],
                                start=True, stop=True,
                            )
                            pt = ptp.tile([128, 512], F32R,
                                          name=f"pt{b}{h}{kb}{qc}", tag="pt")
                            nc.scalar.activation(
                                pt[:], sps[:],
                                mybir.ActivationFunctionType.Exp,
                                scale=0.125,
                            )
                            if qc == kb // 4:
                                nc.vector.tensor_mul(
                                    pt[:], pt[:], mask_sb[kb % 4][:])
                            res.append((qc, pt))
                        return res

                    def av_group(kb, pts, b=b, h=h, avp=avp):
                        for qc, pt in pts:
                            nc.tensor.matmul(
                                avp[:, qc * 512:(qc + 1) * 512],
                                v_sb[b * 16 + kb][:, h * (HD + 1):
                                                  (h + 1) * (HD + 1)],
                                pt[:],
                                start=(kb == 0), stop=(kb == 4 * qc + 3),
                            )

                    # software-pipeline S one kb-group ahead of AV
                    prev = s_group(0)
                    for kb in range(1, 16):
                        cur = s_group(kb)
                        av_group(kb - 1, prev)
                        prev = cur
                    av_group(15, prev)

                    for qc in range(4):
                        qsl = slice(qc * 512, (qc + 1) * 512)
                        den = wk.tile([HD + 1, 512], F32,
                                      name=f"den{b}{h}{qc}", tag="den")
                        nc.scalar.copy(den[HD:HD + 1, :], avp[HD:HD + 1, qsl])
                        den0 = wk.tile([1, 512], F32, name=f"den0{b}{h}{qc}",
                                       tag="den0")
                        nc.sync.dma_start(den0[:], den[HD:HD + 1, :])
                        rcp0 = wk.tile([1, 512], F32, name=f"rcp0{b}{h}{qc}",
                                       tag="rcp0")
                        scr = wk.tile([1, 512], F32, name=f"scr{b}{h}{qc}",
                                      tag="scr")
                        nc.vector.reciprocal_approx_accurate(
                            out=rcp0[:], in_=den0[:], scratch=scr[:])
                        bc = wk.tile([HD, 512], F32, name=f"bc{b}{h}{qc}",
                                     tag="bc")
                        nc.gpsimd.partition_broadcast(bc[:], rcp0[:])
                        if h == 0:
                            nc.vector.tensor_mul(atc[b * 4 + qc][0:HD, :],
                                                 avp[0:HD, qsl], bc[:])
                        else:
                            ath1 = wk.tile([HD, 512], F32R,
                                           name=f"ath1{b}{qc}", tag="ath1")
                            nc.vector.tensor_mul(ath1[:], avp[0:HD, qsl],
                                                 bc[:])
                            nc.sync.dma_start(atc[b * 4 + qc][HD:128, :],
                                              ath1[:])

            # ---- phase 5: partial out-proj per (batch, col-half) + RS ----
            def outproj(b, j):
                osl = slice(j * 512, (j + 1) * 512)
                for tb16 in range(T // 128):
                    tb = b * 16 + tb16
                    po = psA.tile([128, 512], F32, name=f"po{b}{j}{tb16}",
                                  tag="ps_a")
                    nc.tensor.matmul(
                        po[:],
                        atc[tb // 4][:, (tb % 4) * 128:(tb % 4 + 1) * 128],
                        wp_sb[:, osl],
                        start=True, stop=True,
                    )
                    st = wk.tile([128, 512], BF16, name=f"st{b}{j}{tb16}",
                                 tag="st")
                    nc.vector.tensor_copy(st[:], po[:])
                    nc.sync.dma_start(
                        partial_c[b, j][tb16 * 128:(tb16 + 1) * 128, :],
                        st[:])
                nc.gpsimd.collective_compute(
                    "ReduceScatter",
                    mybir.AluOpType.add,
                    replica_groups=[list(range(NC))],
                    ins=[partial_c[b, j][:]],
                    outs=[rs_c[b, j][:]],
                )
                for r in range(T // NC // 128):
                    rsb = wk.tile([128, 512], BF16, name=f"rsb{b}{j}{r}",
                                  tag="rsb")
                    nc.sync.dma_start(rsb[:],
                                      rs_c[b, j][r * 128:(r + 1) * 128, :])
                    rsf = wk.tile([128, 512], F32, name=f"rsf{b}{j}{r}",
                                  tag="rsf")
                    nc.vector.tensor_copy(rsf[:], rsb[:])
                    nc.sync.dma_start(
                        out_d[b * (T // NC) + r * 128:
                              b * (T // NC) + (r + 1) * 128,
                              j * 512:(j + 1) * 512], rsf[:])

            attention(0)
            outproj(0, 0)
            outproj(0, 1)
            attention(1)
            outproj(1, 0)
            outproj(1, 1)

    nc.finalize()
    return nc


def host_inputs(x, w_attn, w_proj):
    """Host-side sharding/layout prep. Returns per-core in_maps."""
    x2 = np.ascontiguousarray(x.reshape(BT, C).T).astype(np.float32)  # [C,BT]

    inv = 1.0 / (ROPE_BASE ** (np.arange(0, HD, 2, dtype=np.float32) / HD))
    tpos = np.arange(T, dtype=np.float32)
    freqs = tpos[:, None] * inv[None, :]                  # [T, 32]
    emb = np.concatenate([freqs, freqs], axis=-1)         # [T, 64]
    cosT = np.cos(emb).T.astype(np.float32)               # [64, T]
    sinT = np.sin(emb).T.astype(np.float32)
    cos_full = np.ascontiguousarray(np.tile(cosT, (2, B)))  # [128, BT]
    sin_full = np.ascontiguousarray(np.tile(sinT, (2, B)))

    m64 = np.zeros((HD, HD), dtype=np.float32)
    half = HD // 2
    for d in range(half):
        m64[d, d + half] = -1.0
        m64[d + half, d] = 1.0
    perm = np.zeros((128, 128), dtype=np.float32)
    perm[0:HD, 0:HD] = m64
    perm[HD:128, HD:128] = m64
    permT = np.ascontiguousarray(perm.T)

    masks = np.zeros((4, 128, 512), dtype=np.float32)
    qi = np.arange(512)[None, :]
    ki = np.arange(128)[:, None]
    for m in range(4):
        masks[m] = (qi - ki >= m * 128).astype(np.float32)

    ident = np.eye(128, dtype=np.float32)

    in_maps = []
    for i in range(NC):
        r0 = i * (HL * HD)
        wq = w_attn[r0:r0 + HL * HD, :]
        wk_ = w_attn[C + r0:C + r0 + HL * HD, :]
        wv = w_attn[2 * C + r0:2 * C + r0 + HL * HD, :]
        wqkvT = np.ascontiguousarray(
            np.concatenate([wq, wk_, wv], axis=0).T).astype(np.float32)
        c0 = i * (HL * HD)
        wpT = np.ascontiguousarray(
            w_proj[:, c0:c0 + 2 * HD].T).astype(np.float32)
        in_maps.append({
            "xT": x2, "wqkvT": wqkvT, "wpT": wpT,
            "cosT": cos_full, "sinT": sin_full, "permT": permT,
            "masks": masks, "ident": ident,
        })
    return in_maps


_NC_CACHE = None


def _get_nc():
    global _NC_CACHE
    if _NC_CACHE is None:
        _NC_CACHE = build()
    return _NC_CACHE


def run(x, w_attn, w_proj, trace=False):
    nc = _get_nc()
    in_maps = host_inputs(np.asarray(x), np.asarray(w_attn),
                          np.asarray(w_proj))
    res = run_bass_kernel_spmd(nc, in_maps, list(range(NC)), trace=trace)
    # core i returns [512, 1024]: rows 0:256 = batch0 rows [256i, 256i+256),
    # rows 256:512 = batch1 rows [256i, 256i+256)
    out = np.empty((B, T, C), dtype=np.float32)
    piece = T // NC
    for i in range(NC):
        sh = res.results[i]["out"]
        out[0, i * piece:(i + 1) * piece] = sh[0:piece]
        out[1, i * piece:(i + 1) * piece] = sh[piece:2 * piece]
    return out, res


def kernel(x, w_attn, w_proj):
    out, _ = run(x, w_attn, w_proj, trace=False)
    return out
